# revision 10
# baseline (speedup 1.0000x reference)
"""BasicVQVAE forward on 8 Trainium2 NeuronCores (Bass/Tile).

Strategy: pure data-parallel over the batch (8192 -> 8 x 1024). Each core runs
the full VQ-VAE forward on its shard; the tiny scalar outputs (vq_loss,
perplexity) are combined on the host from per-core partial sums / indices.

Numerics:
  - encoder chain (x -> h -> z -> z_e) runs in true fp32: the argmin is
    extremely sensitive (near-tie codes flip on ~1e-5 perturbations, and one
    flipped row moves the whole reconstruction row).
  - VQ distances run in fp32r (the PE's fast reduced-precision fp32 mode, 2x
    faster than fp32) with score and index packed into one uint32 so MAX8
    alone carries candidates; the top-4 candidates per row are then re-scored
    exactly in fp32 on the vector engine, which restores exact-argmin quality.
  - decoder runs in bf16 (weights cast on host): output tolerance is smooth
    there (~0.3% relative), and bf16 runs the PE at its fastest rate.

Layout: activations are kept feature-major ([feat, batch]) so every weight
matrix is consumed in its natural [in, out] layout as the stationary lhsT.
x is transposed on the host during shard prep. The last decoder layer swaps
lhsT/rhs (activations stationary, weights moving) to emit batch-major
x_recon directly, so no output transpose is needed anywhere.

The two batch blocks are software-pipelined: block 1's enc1 is issued between
block 0's distance pass and its refine/decode, and block 0's dec2 after block
1's distance pass, so the (in-order) tensor engine never idles on the
refine's gather/DVE latency.
"""

import os
import sys

sys.path.insert(0, "/opt/trn_rl_repo")

import numpy as np
import ml_dtypes

import concourse.bass as bass
import concourse.mybir as mybir
from concourse import bacc
from concourse.tile import TileContext
from concourse.bass_utils import run_bass_kernel_spmd
from concourse.masks import make_identity

F32 = mybir.dt.float32
F32R = mybir.dt.float32r
BF16 = mybir.dt.bfloat16
U32 = mybir.dt.uint32
AF = mybir.ActivationFunctionType
ALU = mybir.AluOpType

B, X, H, Z, E, K = 8192, 4096, 4096, 512, 512, 8192
COMMIT = 0.25
NCORES = 8
BSH = B // NCORES          # 1024 rows per core
NBLK = 2                   # batch blocks per core
BBLK = BSH // NBLK         # 512 rows per block
P = 128
NBT = BBLK // P            # 4 row tiles per block

_CACHE = {}


def _build_program():
    nc = bacc.Bacc("TRN2", target_bir_lowering=False, debug=False,
                   num_devices=NCORES)

    xt = nc.dram_tensor("xt", [X, BSH], F32, kind="ExternalInput").ap()
    w1 = nc.dram_tensor("w1", [X, H], F32, kind="ExternalInput").ap()
    b1 = nc.dram_tensor("b1", [H], F32, kind="ExternalInput").ap()
    w2 = nc.dram_tensor("w2", [H, Z], F32, kind="ExternalInput").ap()
    b2 = nc.dram_tensor("b2", [Z], F32, kind="ExternalInput").ap()
    pw = nc.dram_tensor("pw", [Z, E], F32, kind="ExternalInput").ap()
    pb = nc.dram_tensor("pb", [E], F32, kind="ExternalInput").ap()
    cb2t = nc.dram_tensor("cb2t", [E, K], F32R, kind="ExternalInput").ap()
    cnorm = nc.dram_tensor("cnorm", [K], F32, kind="ExternalInput").ap()
    cb = nc.dram_tensor("cb", [K, E], F32, kind="ExternalInput").ap()
    d1 = nc.dram_tensor("d1", [E, H], BF16, kind="ExternalInput").ap()
    db1 = nc.dram_tensor("db1", [H], F32, kind="ExternalInput").ap()
    d2 = nc.dram_tensor("d2", [H, X], BF16, kind="ExternalInput").ap()
    db2 = nc.dram_tensor("db2", [X], F32, kind="ExternalInput").ap()

    xrec = nc.dram_tensor("xrec", [BSH, X], F32, kind="ExternalOutput").ap()
    idxo = nc.dram_tensor("idxo", [BSH], U32, kind="ExternalOutput").ap()
    losso = nc.dram_tensor("losso", [P, NBLK * NBT], F32,
                           kind="ExternalOutput").ap()

    ht_dram = nc.dram_tensor("ht_dram", [H, BSH], F32).ap()

    XO, HO, ZO, EO, KCC = X // P, H // P, Z // P, E // P, K // 512

    with TileContext(nc) as tc:
        with (
            tc.tile_pool(name="const", bufs=1) as const_pool,
            tc.tile_pool(name="wstream", bufs=3) as wpool,
            tc.tile_pool(name="astream", bufs=3) as apool,
            tc.tile_pool(name="zbuf", bufs=1) as zpool,
            tc.tile_pool(name="hd", bufs=1) as hdpool,
            tc.tile_pool(name="small", bufs=2) as spool,
            tc.tile_pool(name="cbstream", bufs=2) as cbpool,
            tc.tile_pool(name="tiny", bufs=4) as tpool,
            tc.tile_pool(name="psum", bufs=8, space="PSUM") as pp,
        ):
            ident = const_pool.tile([P, P], F32, tag="ident")
            make_identity(nc, ident)
            b1_sb = const_pool.tile([P, HO], F32, tag="b1")
            nc.sync.dma_start(b1_sb, b1.rearrange("(o p) -> p o", p=P))
            b2_sb = const_pool.tile([P, ZO], F32, tag="b2")
            nc.sync.dma_start(b2_sb, b2.rearrange("(o p) -> p o", p=P))
            pb_sb = const_pool.tile([P, EO], F32, tag="pb")
            nc.sync.dma_start(pb_sb, pb.rearrange("(o p) -> p o", p=P))
            db1_sb = const_pool.tile([P, HO], F32, tag="db1")
            nc.sync.dma_start(db1_sb, db1.rearrange("(o p) -> p o", p=P))
            pw_sb = const_pool.tile([P, ZO, E], F32, tag="pw")
            nc.sync.dma_start(pw_sb, pw.rearrange("(ko p) n -> p ko n", p=P))
            ci512 = const_pool.tile([P, 512], U32, tag="ci512")
            nc.gpsimd.iota(ci512, pattern=[[1, 512]], base=0,
                           channel_multiplier=0)

            st = [dict() for _ in range(NBLK)]   # per-block live tiles

            def bslice(b):
                return slice(b * BBLK, (b + 1) * BBLK)

            def enc1(b):
                bsl = bslice(b)
                for mg in range(8):           # groups of 4 H-tiles
                    psums = []
                    for k in range(XO):
                        w1s = wpool.tile([P, 512], F32, tag="w1s")
                        nc.sync.dma_start(
                            w1s, w1[k * P:(k + 1) * P, mg * 512:(mg + 1) * 512])
                        xtc = apool.tile([P, BBLK], F32, tag="xtc")
                        nc.sync.dma_start(xtc, xt[k * P:(k + 1) * P, bsl])
                        if k == 0:
                            psums = [pp.tile([P, BBLK], F32, tag="ps",
                                              name=f"ps_e1_{i}")
                                     for i in range(4)]
                        for m in range(4):
                            nc.tensor.matmul(
                                psums[m], lhsT=w1s[:, m * P:(m + 1) * P],
                                rhs=xtc, start=(k == 0), stop=(k == XO - 1))
                    for m in range(4):
                        hmi = mg * 4 + m
                        hb = apool.tile([P, BBLK], F32, tag="hb")
                        nc.scalar.activation(hb, psums[m], AF.Relu,
                                             bias=b1_sb[:, hmi:hmi + 1])
                        nc.sync.dma_start(
                            ht_dram[hmi * P:(hmi + 1) * P, bsl], hb)

            def enc2_pre(b):
                bsl = bslice(b)
                z_t = zpool.tile([P, ZO, BBLK], F32, tag="zt")
                zps = []
                for k in range(HO):
                    w2s = wpool.tile([P, Z], F32, tag="w2s")
                    nc.sync.dma_start(w2s, w2[k * P:(k + 1) * P, :])
                    htc = apool.tile([P, BBLK], F32, tag="htc")
                    nc.sync.dma_start(htc, ht_dram[k * P:(k + 1) * P, bsl])
                    if k == 0:
                        zps = [pp.tile([P, BBLK], F32, tag="ps",
                                       name=f"ps_e2_{i}")
                               for i in range(ZO)]
                    for m in range(ZO):
                        nc.tensor.matmul(zps[m], lhsT=w2s[:, m * P:(m + 1) * P],
                                         rhs=htc, start=(k == 0),
                                         stop=(k == HO - 1))
                for m in range(ZO):
                    nc.scalar.activation(z_t[:, m, :], zps[m], AF.Identity,
                                         bias=b2_sb[:, m:m + 1])

                ze_t = zpool.tile([P, EO, BBLK], F32, tag="zet")
                eps = [pp.tile([P, BBLK], F32, tag="ps", name=f"ps_pre_{i}")
                       for i in range(EO)]
                for k in range(ZO):
                    for m in range(EO):
                        nc.tensor.matmul(
                            eps[m], lhsT=pw_sb[:, k, m * P:(m + 1) * P],
                            rhs=z_t[:, k, :], start=(k == 0),
                            stop=(k == ZO - 1))
                for m in range(EO):
                    nc.scalar.activation(ze_t[:, m, :], eps[m], AF.Identity,
                                         bias=pb_sb[:, m:m + 1])
                st[b]["ze_t"] = ze_t

            def dist(b):
                # coarse f32r scores; pack (score, index) into uint32 so
                # per-chunk MAX8 alone carries the candidates:
                #   pack = trunc((2z.c - |c|^2)*256 + 1024)*8192 + (8191-idx)
                ze_t = st[b]["ze_t"]
                ze_tr = zpool.tile([P, EO, BBLK], F32R, tag="zetr")
                nc.vector.tensor_copy(ze_tr, ze_t)
                # batch-major z_e for the refine, done early to keep the PE
                # ahead of the DVE/gather chain
                ze_bm = [zpool.tile([P, E], F32, tag=f"zebm{bt}",
                                    name=f"zebm{bt}")
                         for bt in range(NBT)]
                for bt in range(NBT):
                    for e in range(EO):
                        tps = pp.tile([P, P], F32, tag="ps")
                        nc.tensor.transpose(
                            tps, ze_t[:, e, bt * P:(bt + 1) * P], ident)
                        nc.vector.tensor_copy(
                            ze_bm[bt][:, e * P:(e + 1) * P], tps)
                st[b]["ze_bm"] = ze_bm
                candv = [zpool.tile([P, KCC * 8], U32, tag=f"cv{bt}",
                                    name=f"cv{bt}")
                         for bt in range(NBT)]
                for cc in range(KCC):
                    ccsl = slice(cc * 512, (cc + 1) * 512)
                    cbc = cbpool.tile([P, EO, 512], F32R, tag="cbc")
                    nc.scalar.dma_start(
                        cbc, cb2t.rearrange("(ko p) n -> p ko n", p=P)[:, :, ccsl])
                    cnbc = spool.tile([P, 512], F32, tag="cnbc")
                    nc.scalar.dma_start(
                        cnbc,
                        cnorm[ccsl].rearrange("(o n) -> o n", o=1)
                        .broadcast_to([P, 512]))
                    for bt in range(NBT):
                        dps = pp.tile([P, 512], F32, tag="ps")
                        for e in range(EO):
                            nc.tensor.matmul(
                                dps, lhsT=ze_tr[:, e, bt * P:(bt + 1) * P],
                                rhs=cbc[:, e, :], start=(e == 0),
                                stop=(e == EO - 1))
                        # host pre-scales: psum = 256*(2 z.c), cnbc = 1024-256|c|^2
                        s2 = spool.tile([P, 512], F32, tag="s2")
                        nc.vector.tensor_tensor(s2, dps, cnbc, op=ALU.add)
                        su = spool.tile([P, 512], U32, tag="su")
                        nc.gpsimd.tensor_copy(su, s2)
                        pk = spool.tile([P, 512], U32, tag="pk")
                        nc.gpsimd.tensor_scalar(pk, su, 8192, 8191 - cc * 512,
                                                op0=ALU.mult, op1=ALU.add)
                        pk2 = spool.tile([P, 512], U32, tag="pk2")
                        eng = nc.vector if bt % 2 == 0 else nc.gpsimd
                        eng.tensor_tensor(pk2, pk, ci512, op=ALU.subtract)
                        nc.vector.max(out=candv[bt][:, cc * 8:(cc + 1) * 8],
                                      in_=pk2)
                st[b]["candv"] = candv

            def refine(b):
                # exact fp32 re-score of the top-4 candidates per row
                ze_bm = st[b]["ze_bm"]
                candv = st[b]["candv"]
                zq_tb = zpool.tile([P, EO, BBLK], BF16, tag="zqtb")
                idxs, cjs, run_vals, run_idxs = [], [], [], []
                for bt in range(NBT):
                    top8 = tpool.tile([P, 8], U32, tag=f"top8{bt}",
                                      name=f"top8{bt}")
                    nc.vector.max(out=top8, in_=candv[bt])
                    idx4 = tpool.tile([P, 4], U32, tag=f"idx4{bt}",
                                      name=f"idx4{bt}")
                    nc.vector.tensor_scalar(idx4, top8[:, 0:4], 8191, 8191,
                                            op0=ALU.bitwise_and,
                                            op1=ALU.bitwise_xor)
                    idxs.append(idx4)
                for bt in range(NBT):
                    row = []
                    for j in range(4):
                        cj = apool.tile([P, E], F32, tag="zq")
                        nc.gpsimd.indirect_dma_start(
                            out=cj, out_offset=None, in_=cb[:],
                            in_offset=bass.IndirectOffsetOnAxis(
                                ap=idxs[bt][:, j:j + 1], axis=0))
                        row.append(cj)
                    cjs.append(row)
                for bt in range(NBT):
                    run_val = tpool.tile([P, 1], F32, tag=f"rv{bt}",
                                         name=f"rv{bt}")
                    run_idx = tpool.tile([P, 1], U32, tag=f"ri{bt}",
                                         name=f"ri{bt}")
                    nc.vector.memset(run_val, -3.0e38)
                    nc.vector.memset(run_idx, 0)
                    run_vals.append(run_val)
                    run_idxs.append(run_idx)
                    for j in range(4):
                        cj = cjs[bt][j]
                        idx_j = idxs[bt][:, j:j + 1]
                        # exact score = sum(c * (2z - c))
                        t1 = spool.tile([P, E], F32, tag="t1")
                        nc.vector.scalar_tensor_tensor(
                            t1, ze_bm[bt], 2.0, cj, op0=ALU.mult,
                            op1=ALU.subtract)
                        t2 = spool.tile([P, E], F32, tag="t2")
                        nc.vector.tensor_tensor(t2, t1, cj, op=ALU.mult)
                        exj = tpool.tile([P, 1], F32, tag="exj")
                        nc.vector.reduce_sum(exj, t2, axis=mybir.AxisListType.X)
                        m_gt = tpool.tile([P, 1], U32, tag="mgt")
                        nc.vector.tensor_tensor(m_gt, exj, run_val,
                                                op=ALU.is_gt)
                        m_eq = tpool.tile([P, 1], U32, tag="meq")
                        nc.vector.tensor_tensor(m_eq, exj, run_val,
                                                op=ALU.is_equal)
                        m_lt = tpool.tile([P, 1], U32, tag="mlt")
                        nc.vector.tensor_tensor(m_lt, idx_j, run_idx,
                                                op=ALU.is_lt)
                        m_tie = tpool.tile([P, 1], U32, tag="mtie")
                        nc.vector.tensor_tensor(m_tie, m_eq, m_lt,
                                                op=ALU.logical_and)
                        upd = tpool.tile([P, 1], U32, tag="upd")
                        nc.vector.tensor_tensor(upd, m_gt, m_tie,
                                                op=ALU.logical_or)
                        nc.vector.copy_predicated(run_val, upd, exj)
                        nc.vector.copy_predicated(run_idx, upd, idx_j)

                zq_bt = []
                for bt in range(NBT):
                    gbase = b * BBLK + bt * P
                    nc.sync.dma_start(
                        idxo[gbase:gbase + P].rearrange("(n o) -> n o", o=1),
                        run_idxs[bt])
                    zq = apool.tile([P, E], F32, tag="zq")
                    nc.gpsimd.indirect_dma_start(
                        out=zq, out_offset=None, in_=cb[:],
                        in_offset=bass.IndirectOffsetOnAxis(
                            ap=run_idxs[bt][:, 0:1], axis=0))
                    zq_bt.append(zq)
                    # loss partials: sum((z_q - z_e)^2) over this row tile
                    ld = spool.tile([P, E], F32, tag="t1")
                    nc.vector.tensor_sub(ld, zq, ze_bm[bt])
                    lsq = spool.tile([P, E], F32, tag="t2")
                    lacc = tpool.tile([P, 1], F32, tag="lacc")
                    nc.scalar.activation(lsq, ld, AF.Square, accum_out=lacc)
                    nc.sync.dma_start(
                        losso[:, b * NBT + bt:b * NBT + bt + 1], lacc)
                for bt in range(NBT):
                    for e in range(EO):
                        tps = pp.tile([P, P], F32, tag="ps")
                        nc.tensor.transpose(
                            tps, zq_bt[bt][:, e * P:(e + 1) * P], ident)
                        nc.vector.tensor_copy(
                            zq_tb[:, e, bt * P:(bt + 1) * P], tps)
                st[b]["zq_tb"] = zq_tb

            def dec1(b):
                zq_tb = st[b]["zq_tb"]
                hd_t = hdpool.tile([P, HO, BBLK], BF16, tag="hdt")
                for mg in range(8):
                    d1s = wpool.tile([P, EO, 512], BF16, tag="d1s")
                    nc.sync.dma_start(
                        d1s, d1.rearrange("(ko p) n -> p ko n", p=P)
                        [:, :, mg * 512:(mg + 1) * 512])
                    for m in range(4):
                        hmi = mg * 4 + m
                        dps = pp.tile([P, BBLK], F32, tag="ps")
                        for k in range(EO):
                            nc.tensor.matmul(
                                dps, lhsT=d1s[:, k, m * P:(m + 1) * P],
                                rhs=zq_tb[:, k, :], start=(k == 0),
                                stop=(k == EO - 1))
                        nc.scalar.activation(hd_t[:, hmi, :], dps, AF.Relu,
                                             bias=db1_sb[:, hmi:hmi + 1])
                st[b]["hd_t"] = hd_t

            def dec2(b):
                hd_t = st[b]["hd_t"]
                for xo in range(8):
                    xsl = slice(xo * 512, (xo + 1) * 512)
                    b2bc = spool.tile([P, 512], F32, tag="b2bc")
                    nc.sync.dma_start(
                        b2bc,
                        db2[xsl].rearrange("(o n) -> o n", o=1)
                        .broadcast_to([P, 512]))
                    ops = [pp.tile([P, 512], F32, tag="ps",
                                   name=f"ps_d2_{i}")
                           for i in range(NBT)]
                    for ho in range(HO):
                        w2c = wpool.tile([P, 512], BF16, tag="w2c")
                        nc.sync.dma_start(w2c, d2[ho * P:(ho + 1) * P, xsl])
                        for bt in range(NBT):
                            nc.tensor.matmul(
                                ops[bt], lhsT=hd_t[:, ho, bt * P:(bt + 1) * P],
                                rhs=w2c, start=(ho == 0), stop=(ho == HO - 1))
                    for bt in range(NBT):
                        osb = apool.tile([P, 512], F32, tag="osb")
                        nc.vector.tensor_tensor(osb, ops[bt], b2bc, op=ALU.add)
                        rbase = b * BBLK + bt * P
                        nc.sync.dma_start(xrec[rbase:rbase + P, xsl], osb)

            # software pipeline over the two blocks: keep the in-order PE fed
            # during the refine's gather/DVE latency
            enc1(0)
            enc2_pre(0)
            dist(0)
            enc1(1)
            refine(0)
            dec1(0)
            enc2_pre(1)
            dist(1)
            dec2(0)
            refine(1)
            dec1(1)
            dec2(1)

    nc.compile()
    return nc


def kernel(**inputs):
    inp = {k: np.asarray(v) for k, v in inputs.items()}
    x = inp["x"].astype(np.float32, copy=False)
    codebook = inp["codebook"].astype(np.float32, copy=False)

    if "nc" not in _CACHE:
        _CACHE["nc"] = _build_program()
    nc = _CACHE["nc"]

    cb2t = np.ascontiguousarray((512.0 * codebook).T)
    cnorm = (1024.0 - 256.0 * (codebook.astype(np.float64) ** 2).sum(axis=1)).astype(np.float32)
    d1_bf = inp["dec_w1"].astype(ml_dtypes.bfloat16)
    d2_bf = inp["dec_w2"].astype(ml_dtypes.bfloat16)

    shared = {
        "w1": np.ascontiguousarray(inp["enc_w1"], dtype=np.float32),
        "b1": inp["enc_b1"].astype(np.float32, copy=False),
        "w2": np.ascontiguousarray(inp["enc_w2"], dtype=np.float32),
        "b2": inp["enc_b2"].astype(np.float32, copy=False),
        "pw": np.ascontiguousarray(inp["pre_w"], dtype=np.float32),
        "pb": inp["pre_b"].astype(np.float32, copy=False),
        "cb2t": cb2t,
        "cnorm": cnorm,
        "cb": np.ascontiguousarray(codebook),
        "d1": d1_bf,
        "db1": inp["dec_b1"].astype(np.float32, copy=False),
        "d2": d2_bf,
        "db2": inp["dec_b2"].astype(np.float32, copy=False),
    }
    in_maps = []
    for c in range(NCORES):
        xt_sh = np.ascontiguousarray(x[c * BSH:(c + 1) * BSH].T)
        in_maps.append({**shared, "xt": xt_sh})

    trace = bool(os.environ.get("BASS_TRACE"))
    results = run_bass_kernel_spmd(
        nc, in_maps, list(range(NCORES)), trace=trace,
        tmpdir=os.environ.get("KERNEL_TRACE_DIR") or None)
    _CACHE["last_results"] = results

    x_recon = np.empty((B, X), dtype=np.float32)
    idx_all = np.empty((B,), dtype=np.int64)
    loss_total = 0.0
    for c in range(NCORES):
        r = results.results[c]
        x_recon[c * BSH:(c + 1) * BSH] = r["xrec"]
        idx_all[c * BSH:(c + 1) * BSH] = r["idxo"].astype(np.int64)
        loss_total += r["losso"].astype(np.float64).sum()

    mse = loss_total / (B * E)
    vq_loss = np.float32((1.0 + COMMIT) * mse)

    counts = np.bincount(idx_all, minlength=K).astype(np.float32)
    avg = counts / np.float32(B)
    perplexity = np.float32(
        np.exp(-np.sum(avg * np.log(avg + 1e-10), dtype=np.float64)))

    return (x_recon, vq_loss, perplexity)


# revision 14
# speedup vs baseline: 1.0913x; 1.0913x over previous
"""BasicVQVAE forward on 8 Trainium2 NeuronCores (Bass/Tile).

Strategy: pure data-parallel over the batch (8192 -> 8 x 1024). Each core runs
the full VQ-VAE forward on its shard; the tiny scalar outputs (vq_loss,
perplexity) are combined on the host from per-core partial sums / indices.

Numerics:
  - encoder chain (x -> h -> z -> z_e) runs in true fp32: the argmin is
    extremely sensitive (near-tie codes flip on ~1e-5 perturbations, and one
    flipped row moves the whole reconstruction row).
  - VQ distances run in fp32r (the PE's fast reduced-precision fp32 mode, 2x
    faster than fp32) with score and index packed into one uint32 so MAX8
    alone carries candidates; the top-4 candidates per row are then re-scored
    exactly in fp32 on the vector engine, which restores exact-argmin quality.
  - decoder runs in bf16 (weights cast on host): output tolerance is smooth
    there (~0.3% relative), and bf16 runs the PE at its fastest rate.

Layout: activations are kept feature-major ([feat, batch]) so every weight
matrix is consumed in its natural [in, out] layout as the stationary lhsT.
x is transposed on the host during shard prep. The last decoder layer swaps
lhsT/rhs (activations stationary, weights moving) to emit batch-major
x_recon directly, so no output transpose is needed anywhere.

The two batch blocks are software-pipelined: block 1's enc1 is issued between
block 0's distance pass and its refine/decode, and block 0's dec2 after block
1's distance pass, so the (in-order) tensor engine never idles on the
refine's gather/DVE latency.
"""

import os
import sys

sys.path.insert(0, "/opt/trn_rl_repo")

import numpy as np
import ml_dtypes

import concourse.bass as bass
import concourse.mybir as mybir
from concourse import bacc
from concourse.tile import TileContext
from concourse.bass_utils import run_bass_kernel_spmd
from concourse.masks import make_identity

F32 = mybir.dt.float32
F32R = mybir.dt.float32r
BF16 = mybir.dt.bfloat16
U32 = mybir.dt.uint32
AF = mybir.ActivationFunctionType
ALU = mybir.AluOpType

B, X, H, Z, E, K = 8192, 4096, 4096, 512, 512, 8192
COMMIT = 0.25
NCORES = 8
BSH = B // NCORES          # 1024 rows per core
NBLK = 2                   # batch blocks per core
BBLK = BSH // NBLK         # 512 rows per block
P = 128
NBT = BBLK // P            # 4 row tiles per block

_CACHE = {}


def _build_program():
    nc = bacc.Bacc("TRN2", target_bir_lowering=False, debug=False,
                   num_devices=NCORES)

    xt = nc.dram_tensor("xt", [X, BSH], F32, kind="ExternalInput").ap()
    w1 = nc.dram_tensor("w1", [X, H], F32, kind="ExternalInput").ap()
    b1 = nc.dram_tensor("b1", [H], F32, kind="ExternalInput").ap()
    w2 = nc.dram_tensor("w2", [H, Z], F32, kind="ExternalInput").ap()
    b2 = nc.dram_tensor("b2", [Z], F32, kind="ExternalInput").ap()
    pw = nc.dram_tensor("pw", [Z, E], F32, kind="ExternalInput").ap()
    pb = nc.dram_tensor("pb", [E], F32, kind="ExternalInput").ap()
    cb2t = nc.dram_tensor("cb2t", [E, K], F32R, kind="ExternalInput").ap()
    cnorm = nc.dram_tensor("cnorm", [K], F32, kind="ExternalInput").ap()
    cb = nc.dram_tensor("cb", [K, E], F32, kind="ExternalInput").ap()
    d1 = nc.dram_tensor("d1", [E, H], BF16, kind="ExternalInput").ap()
    db1 = nc.dram_tensor("db1", [H], F32, kind="ExternalInput").ap()
    d2 = nc.dram_tensor("d2", [H, X], BF16, kind="ExternalInput").ap()
    db2 = nc.dram_tensor("db2", [X], F32, kind="ExternalInput").ap()

    xrec = nc.dram_tensor("xrec", [BSH, X], F32, kind="ExternalOutput").ap()
    idxo = nc.dram_tensor("idxo", [BSH], U32, kind="ExternalOutput").ap()
    losso = nc.dram_tensor("losso", [P, NBLK * NBT], F32,
                           kind="ExternalOutput").ap()

    ht_dram = nc.dram_tensor("ht_dram", [H, BSH], F32).ap()

    XO, HO, ZO, EO, KCC = X // P, H // P, Z // P, E // P, K // 512

    with TileContext(nc) as tc:
        with (
            tc.tile_pool(name="const", bufs=1) as const_pool,
            tc.tile_pool(name="wstream", bufs=3) as wpool,
            tc.tile_pool(name="astream", bufs=3) as apool,
            tc.tile_pool(name="zbuf", bufs=1) as zpool,
            tc.tile_pool(name="hd", bufs=1) as hdpool,
            tc.tile_pool(name="small", bufs=2) as spool,
            tc.tile_pool(name="cbstream", bufs=2) as cbpool,
            tc.tile_pool(name="tiny", bufs=4) as tpool,
            tc.tile_pool(name="psum", bufs=8, space="PSUM") as pp,
        ):
            ident = const_pool.tile([P, P], F32, tag="ident")
            make_identity(nc, ident)
            b1_sb = const_pool.tile([P, HO], F32, tag="b1")
            nc.sync.dma_start(b1_sb, b1.rearrange("(o p) -> p o", p=P))
            b2_sb = const_pool.tile([P, ZO], F32, tag="b2")
            nc.sync.dma_start(b2_sb, b2.rearrange("(o p) -> p o", p=P))
            pb_sb = const_pool.tile([P, EO], F32, tag="pb")
            nc.sync.dma_start(pb_sb, pb.rearrange("(o p) -> p o", p=P))
            db1_sb = const_pool.tile([P, HO], F32, tag="db1")
            nc.sync.dma_start(db1_sb, db1.rearrange("(o p) -> p o", p=P))
            pw_sb = const_pool.tile([P, ZO, E], F32, tag="pw")
            nc.sync.dma_start(pw_sb, pw.rearrange("(ko p) n -> p ko n", p=P))
            ci512 = const_pool.tile([P, 512], U32, tag="ci512")
            nc.gpsimd.iota(ci512, pattern=[[1, 512]], base=0,
                           channel_multiplier=0)
            c8191 = const_pool.tile([P, 512], U32, tag="c8191")
            nc.vector.memset(c8191, 8191)
            ciob = const_pool.tile([P, 512], U32, tag="ciob")
            nc.vector.tensor_tensor(ciob, c8191, ci512, op=ALU.subtract)

            st = [dict() for _ in range(NBLK)]   # per-block live tiles

            def bslice(b):
                return slice(b * BBLK, (b + 1) * BBLK)

            def enc1_mg(b, mg):
                bsl = bslice(b)
                if True:
                    psums = []
                    for k in range(XO):
                        w1s = wpool.tile([P, 512], F32, tag="w1s")
                        nc.sync.dma_start(
                            w1s, w1[k * P:(k + 1) * P, mg * 512:(mg + 1) * 512])
                        xtc = apool.tile([P, BBLK], F32, tag="xtc")
                        nc.sync.dma_start(xtc, xt[k * P:(k + 1) * P, bsl])
                        if k == 0:
                            psums = [pp.tile([P, BBLK], F32, tag="ps",
                                              name=f"ps_e1_{i}")
                                     for i in range(4)]
                        for m in range(4):
                            nc.tensor.matmul(
                                psums[m], lhsT=w1s[:, m * P:(m + 1) * P],
                                rhs=xtc, start=(k == 0), stop=(k == XO - 1))
                    for m in range(4):
                        hmi = mg * 4 + m
                        hb = apool.tile([P, BBLK], F32, tag="hb")
                        nc.scalar.activation(hb, psums[m], AF.Relu,
                                             bias=b1_sb[:, hmi:hmi + 1])
                        nc.sync.dma_start(
                            ht_dram[hmi * P:(hmi + 1) * P, bsl], hb)

            def enc1(b):
                for mg in range(8):           # groups of 4 H-tiles
                    enc1_mg(b, mg)

            def enc2_pre(b):
                bsl = bslice(b)
                z_t = zpool.tile([P, ZO, BBLK], F32, tag="zt")
                zps = []
                for k in range(HO):
                    w2s = wpool.tile([P, Z], F32, tag="w2s")
                    nc.sync.dma_start(w2s, w2[k * P:(k + 1) * P, :])
                    htc = apool.tile([P, BBLK], F32, tag="htc")
                    nc.sync.dma_start(htc, ht_dram[k * P:(k + 1) * P, bsl])
                    if k == 0:
                        zps = [pp.tile([P, BBLK], F32, tag="ps",
                                       name=f"ps_e2_{i}")
                               for i in range(ZO)]
                    for m in range(ZO):
                        nc.tensor.matmul(zps[m], lhsT=w2s[:, m * P:(m + 1) * P],
                                         rhs=htc, start=(k == 0),
                                         stop=(k == HO - 1))
                for m in range(ZO):
                    nc.scalar.activation(z_t[:, m, :], zps[m], AF.Identity,
                                         bias=b2_sb[:, m:m + 1])

                ze_t = zpool.tile([P, EO, BBLK], F32, tag="zet")
                eps = [pp.tile([P, BBLK], F32, tag="ps", name=f"ps_pre_{i}")
                       for i in range(EO)]
                for k in range(ZO):
                    for m in range(EO):
                        nc.tensor.matmul(
                            eps[m], lhsT=pw_sb[:, k, m * P:(m + 1) * P],
                            rhs=z_t[:, k, :], start=(k == 0),
                            stop=(k == ZO - 1))
                for m in range(EO):
                    nc.scalar.activation(ze_t[:, m, :], eps[m], AF.Identity,
                                         bias=pb_sb[:, m:m + 1])
                st[b]["ze_t"] = ze_t

            def dist_setup(b):
                # coarse f32r scores; pack (score, index) into uint32 so
                # per-chunk MAX8 alone carries the candidates:
                #   pack = trunc((2z.c - |c|^2)*256 + 1024)*8192 + (8191-idx)
                ze_t = st[b]["ze_t"]
                ze_tr = zpool.tile([P, EO, BBLK], F32R, tag="zetr")
                nc.vector.tensor_copy(ze_tr, ze_t)
                # batch-major z_e for the refine, done early to keep the PE
                # ahead of the DVE/gather chain
                ze_bm = [zpool.tile([P, E], F32, tag=f"zebm{bt}",
                                    name=f"zebm{bt}")
                         for bt in range(NBT)]
                for bt in range(NBT):
                    for e in range(EO):
                        tps = pp.tile([P, P], F32, tag="ps")
                        nc.tensor.transpose(
                            tps, ze_t[:, e, bt * P:(bt + 1) * P], ident)
                        nc.vector.tensor_copy(
                            ze_bm[bt][:, e * P:(e + 1) * P], tps)
                st[b]["ze_bm"] = ze_bm
                st[b]["ze_tr"] = ze_tr
                candv = [zpool.tile([P, KCC * 8], U32, tag=f"cv{bt}",
                                    name=f"cv{bt}")
                         for bt in range(NBT)]
                st[b]["candv"] = candv

            def dist_cc(b, cc):
                ze_tr = st[b]["ze_tr"]
                candv = st[b]["candv"]
                if True:
                    ccsl = slice(cc * 512, (cc + 1) * 512)
                    cbc = cbpool.tile([P, EO, 512], F32R, tag="cbc")
                    nc.scalar.dma_start(
                        cbc, cb2t.rearrange("(ko p) n -> p ko n", p=P)[:, :, ccsl])
                    cnbc = spool.tile([P, 512], F32, tag="cnbc")
                    nc.scalar.dma_start(
                        cnbc,
                        cnorm[ccsl].rearrange("(o n) -> o n", o=1)
                        .broadcast_to([P, 512]))
                    cio = spool.tile([P, 512], U32, tag="cio")
                    nc.vector.tensor_scalar(
                        cio, ciob, 512 * cc, None, op0=ALU.subtract)
                    for bt in range(NBT):
                        dps = pp.tile([P, 512], F32, tag="ps")
                        for e in range(EO):
                            nc.tensor.matmul(
                                dps, lhsT=ze_tr[:, e, bt * P:(bt + 1) * P],
                                rhs=cbc[:, e, :], start=(e == 0),
                                stop=(e == EO - 1))
                        # host pre-scales: psum = 256*(2 z.c), cnbc = 1024-256|c|^2
                        # add + truncate-to-uint in one DVE op
                        su = spool.tile([P, 512], U32, tag="su")
                        nc.vector.tensor_tensor(su, dps, cnbc, op=ALU.add)
                        pk2 = spool.tile([P, 512], U32, tag="pk2")
                        nc.vector.scalar_tensor_tensor(
                            pk2, su, 8192, cio, op0=ALU.mult, op1=ALU.add)
                        nc.vector.max(out=candv[bt][:, cc * 8:(cc + 1) * 8],
                                      in_=pk2)

            def refine(b):
                # exact fp32 re-score of the top-4 candidates per row
                ze_bm = st[b]["ze_bm"]
                candv = st[b]["candv"]
                zq_tb = zpool.tile([P, EO, BBLK], BF16, tag="zqtb")
                idxs, cjs, run_vals, run_idxs = [], [], [], []
                for bt in range(NBT):
                    top8 = tpool.tile([P, 8], U32, tag=f"top8{bt}",
                                      name=f"top8{bt}")
                    nc.vector.max(out=top8, in_=candv[bt])
                    idx4 = tpool.tile([P, 4], U32, tag=f"idx4{bt}",
                                      name=f"idx4{bt}")
                    nc.vector.tensor_scalar(idx4, top8[:, 0:4], 8191, 8191,
                                            op0=ALU.bitwise_and,
                                            op1=ALU.bitwise_xor)
                    idxs.append(idx4)
                for bt in range(NBT):
                    row = []
                    for j in range(4):
                        cj = apool.tile([P, E], F32, tag="zq")
                        nc.gpsimd.indirect_dma_start(
                            out=cj, out_offset=None, in_=cb[:],
                            in_offset=bass.IndirectOffsetOnAxis(
                                ap=idxs[bt][:, j:j + 1], axis=0))
                        row.append(cj)
                    cjs.append(row)
                for bt in range(NBT):
                    run_val = tpool.tile([P, 1], F32, tag=f"rv{bt}",
                                         name=f"rv{bt}")
                    run_idx = tpool.tile([P, 1], U32, tag=f"ri{bt}",
                                         name=f"ri{bt}")
                    nc.vector.memset(run_val, -3.0e38)
                    nc.vector.memset(run_idx, 0)
                    run_vals.append(run_val)
                    run_idxs.append(run_idx)
                    for j in range(4):
                        cj = cjs[bt][j]
                        idx_j = idxs[bt][:, j:j + 1]
                        # exact score = sum(c * (2z - c))
                        t1 = spool.tile([P, E], F32, tag="t1")
                        nc.vector.scalar_tensor_tensor(
                            t1, ze_bm[bt], 2.0, cj, op0=ALU.mult,
                            op1=ALU.subtract)
                        t2 = spool.tile([P, E], F32, tag="t2")
                        nc.vector.tensor_tensor(t2, t1, cj, op=ALU.mult)
                        exj = tpool.tile([P, 1], F32, tag="exj")
                        nc.vector.reduce_sum(exj, t2, axis=mybir.AxisListType.X)
                        m_gt = tpool.tile([P, 1], U32, tag="mgt")
                        nc.vector.tensor_tensor(m_gt, exj, run_val,
                                                op=ALU.is_gt)
                        m_eq = tpool.tile([P, 1], U32, tag="meq")
                        nc.vector.tensor_tensor(m_eq, exj, run_val,
                                                op=ALU.is_equal)
                        m_lt = tpool.tile([P, 1], U32, tag="mlt")
                        nc.vector.tensor_tensor(m_lt, idx_j, run_idx,
                                                op=ALU.is_lt)
                        m_tie = tpool.tile([P, 1], U32, tag="mtie")
                        nc.vector.tensor_tensor(m_tie, m_eq, m_lt,
                                                op=ALU.logical_and)
                        upd = tpool.tile([P, 1], U32, tag="upd")
                        nc.vector.tensor_tensor(upd, m_gt, m_tie,
                                                op=ALU.logical_or)
                        nc.vector.copy_predicated(run_val, upd, exj)
                        nc.vector.copy_predicated(run_idx, upd, idx_j)

                zq_bt = []
                for bt in range(NBT):
                    gbase = b * BBLK + bt * P
                    nc.sync.dma_start(
                        idxo[gbase:gbase + P].rearrange("(n o) -> n o", o=1),
                        run_idxs[bt])
                    zq = apool.tile([P, E], F32, tag="zq")
                    nc.gpsimd.indirect_dma_start(
                        out=zq, out_offset=None, in_=cb[:],
                        in_offset=bass.IndirectOffsetOnAxis(
                            ap=run_idxs[bt][:, 0:1], axis=0))
                    zq_bt.append(zq)
                    # loss partials: sum((z_q - z_e)^2) over this row tile
                    ld = spool.tile([P, E], F32, tag="t1")
                    nc.vector.tensor_sub(ld, zq, ze_bm[bt])
                    lsq = spool.tile([P, E], F32, tag="t2")
                    lacc = tpool.tile([P, 1], F32, tag="lacc")
                    nc.scalar.activation(lsq, ld, AF.Square, accum_out=lacc)
                    nc.sync.dma_start(
                        losso[:, b * NBT + bt:b * NBT + bt + 1], lacc)
                for bt in range(NBT):
                    for e in range(EO):
                        tps = pp.tile([P, P], F32, tag="ps")
                        nc.tensor.transpose(
                            tps, zq_bt[bt][:, e * P:(e + 1) * P], ident)
                        nc.vector.tensor_copy(
                            zq_tb[:, e, bt * P:(bt + 1) * P], tps)
                st[b]["zq_tb"] = zq_tb

            def dec1(b):
                zq_tb = st[b]["zq_tb"]
                hd_t = hdpool.tile([P, HO, BBLK], BF16, tag="hdt")
                for mg in range(8):
                    d1s = wpool.tile([P, EO, 512], BF16, tag="d1s")
                    nc.sync.dma_start(
                        d1s, d1.rearrange("(ko p) n -> p ko n", p=P)
                        [:, :, mg * 512:(mg + 1) * 512])
                    for m in range(4):
                        hmi = mg * 4 + m
                        dps = pp.tile([P, BBLK], F32, tag="ps")
                        for k in range(EO):
                            nc.tensor.matmul(
                                dps, lhsT=d1s[:, k, m * P:(m + 1) * P],
                                rhs=zq_tb[:, k, :], start=(k == 0),
                                stop=(k == EO - 1))
                        nc.scalar.activation(hd_t[:, hmi, :], dps, AF.Relu,
                                             bias=db1_sb[:, hmi:hmi + 1])
                st[b]["hd_t"] = hd_t

            def dec2_xo(b, xo):
                hd_t = st[b]["hd_t"]
                if True:
                    xsl = slice(xo * 512, (xo + 1) * 512)
                    b2bc = spool.tile([P, 512], F32, tag="b2bc")
                    nc.sync.dma_start(
                        b2bc,
                        db2[xsl].rearrange("(o n) -> o n", o=1)
                        .broadcast_to([P, 512]))
                    ops = [pp.tile([P, 512], F32, tag="ps",
                                   name=f"ps_d2_{i}")
                           for i in range(NBT)]
                    for ho in range(HO):
                        w2c = wpool.tile([P, 512], BF16, tag="w2c")
                        nc.sync.dma_start(w2c, d2[ho * P:(ho + 1) * P, xsl])
                        for bt in range(NBT):
                            nc.tensor.matmul(
                                ops[bt], lhsT=hd_t[:, ho, bt * P:(bt + 1) * P],
                                rhs=w2c, start=(ho == 0), stop=(ho == HO - 1))
                    for bt in range(NBT):
                        osb = apool.tile([P, 512], F32, tag="osb")
                        nc.vector.tensor_tensor(osb, ops[bt], b2bc, op=ALU.add)
                        rbase = b * BBLK + bt * P
                        nc.sync.dma_start(xrec[rbase:rbase + P, xsl], osb)

            def dec2(b):
                for xo in range(8):
                    dec2_xo(b, xo)

            # software pipeline over the two blocks: keep the in-order PE fed
            # during the refine's gather/DVE latency
            enc1(0)
            enc2_pre(0)
            dist_setup(0)
            for mg in range(8):
                if mg < 4:
                    for cc in range(4 * mg, 4 * mg + 4):
                        dist_cc(0, cc)
                enc1_mg(1, mg)
            refine(0)
            dec1(0)
            enc2_pre(1)
            dist_setup(1)
            for xo in range(8):
                if xo < 4:
                    for cc in range(4 * xo, 4 * xo + 4):
                        dist_cc(1, cc)
                dec2_xo(0, xo)
            refine(1)
            dec1(1)
            dec2(1)

    nc.compile()
    return nc


def kernel(**inputs):
    inp = {k: np.asarray(v) for k, v in inputs.items()}
    x = inp["x"].astype(np.float32, copy=False)
    codebook = inp["codebook"].astype(np.float32, copy=False)

    if "nc" not in _CACHE:
        _CACHE["nc"] = _build_program()
    nc = _CACHE["nc"]

    cb2t = np.ascontiguousarray((512.0 * codebook).T)
    cnorm = (1024.0 - 256.0 * (codebook.astype(np.float64) ** 2).sum(axis=1)).astype(np.float32)
    d1_bf = inp["dec_w1"].astype(ml_dtypes.bfloat16)
    d2_bf = inp["dec_w2"].astype(ml_dtypes.bfloat16)

    shared = {
        "w1": np.ascontiguousarray(inp["enc_w1"], dtype=np.float32),
        "b1": inp["enc_b1"].astype(np.float32, copy=False),
        "w2": np.ascontiguousarray(inp["enc_w2"], dtype=np.float32),
        "b2": inp["enc_b2"].astype(np.float32, copy=False),
        "pw": np.ascontiguousarray(inp["pre_w"], dtype=np.float32),
        "pb": inp["pre_b"].astype(np.float32, copy=False),
        "cb2t": cb2t,
        "cnorm": cnorm,
        "cb": np.ascontiguousarray(codebook),
        "d1": d1_bf,
        "db1": inp["dec_b1"].astype(np.float32, copy=False),
        "d2": d2_bf,
        "db2": inp["dec_b2"].astype(np.float32, copy=False),
    }
    in_maps = []
    for c in range(NCORES):
        xt_sh = np.ascontiguousarray(x[c * BSH:(c + 1) * BSH].T)
        in_maps.append({**shared, "xt": xt_sh})

    trace = bool(os.environ.get("BASS_TRACE"))
    results = run_bass_kernel_spmd(
        nc, in_maps, list(range(NCORES)), trace=trace,
        tmpdir=os.environ.get("KERNEL_TRACE_DIR") or None)
    _CACHE["last_results"] = results

    x_recon = np.empty((B, X), dtype=np.float32)
    idx_all = np.empty((B,), dtype=np.int64)
    loss_total = 0.0
    for c in range(NCORES):
        r = results.results[c]
        x_recon[c * BSH:(c + 1) * BSH] = r["xrec"]
        idx_all[c * BSH:(c + 1) * BSH] = r["idxo"].astype(np.int64)
        loss_total += r["losso"].astype(np.float64).sum()

    mse = loss_total / (B * E)
    vq_loss = np.float32((1.0 + COMMIT) * mse)

    counts = np.bincount(idx_all, minlength=K).astype(np.float32)
    avg = counts / np.float32(B)
    perplexity = np.float32(
        np.exp(-np.sum(avg * np.log(avg + 1e-10), dtype=np.float64)))

    return (x_recon, vq_loss, perplexity)


# revision 15
# speedup vs baseline: 1.1055x; 1.0130x over previous
"""BasicVQVAE forward on 8 Trainium2 NeuronCores (Bass/Tile).

Strategy: pure data-parallel over the batch (8192 -> 8 x 1024). Each core runs
the full VQ-VAE forward on its shard; the tiny scalar outputs (vq_loss,
perplexity) are combined on the host from per-core partial sums / indices.

Numerics:
  - encoder chain (x -> h -> z -> z_e) runs in true fp32: the argmin is
    extremely sensitive (near-tie codes flip on ~1e-5 perturbations, and one
    flipped row moves the whole reconstruction row).
  - VQ distances run in fp32r (the PE's fast reduced-precision fp32 mode, 2x
    faster than fp32) with score and index packed into one uint32 so MAX8
    alone carries candidates; the top-4 candidates per row are then re-scored
    exactly in fp32 on the vector engine, which restores exact-argmin quality.
  - decoder runs in bf16 (weights cast on host): output tolerance is smooth
    there (~0.3% relative), and bf16 runs the PE at its fastest rate.

Layout: activations are kept feature-major ([feat, batch]) so every weight
matrix is consumed in its natural [in, out] layout as the stationary lhsT.
x is transposed on the host during shard prep. The last decoder layer swaps
lhsT/rhs (activations stationary, weights moving) to emit batch-major
x_recon directly, so no output transpose is needed anywhere.

The two batch blocks are software-pipelined: block 1's enc1 is issued between
block 0's distance pass and its refine/decode, and block 0's dec2 after block
1's distance pass, so the (in-order) tensor engine never idles on the
refine's gather/DVE latency.
"""

import os
import sys

sys.path.insert(0, "/opt/trn_rl_repo")

import numpy as np
import ml_dtypes

import concourse.bass as bass
import concourse.mybir as mybir
from concourse import bacc
from concourse.tile import TileContext
from concourse.bass_utils import run_bass_kernel_spmd
from concourse.masks import make_identity

F32 = mybir.dt.float32
F32R = mybir.dt.float32r
BF16 = mybir.dt.bfloat16
U32 = mybir.dt.uint32
AF = mybir.ActivationFunctionType
ALU = mybir.AluOpType

B, X, H, Z, E, K = 8192, 4096, 4096, 512, 512, 8192
COMMIT = 0.25
NCORES = 8
BSH = B // NCORES          # 1024 rows per core
NBLK = 2                   # batch blocks per core
BBLK = BSH // NBLK         # 512 rows per block
P = 128
NBT = BBLK // P            # 4 row tiles per block

_CACHE = {}


def _build_program():
    nc = bacc.Bacc("TRN2", target_bir_lowering=False, debug=False,
                   num_devices=NCORES)

    xt = nc.dram_tensor("xt", [X, BSH], F32, kind="ExternalInput").ap()
    w1 = nc.dram_tensor("w1", [X, H], F32, kind="ExternalInput").ap()
    b1 = nc.dram_tensor("b1", [H], F32, kind="ExternalInput").ap()
    w2 = nc.dram_tensor("w2", [H, Z], F32, kind="ExternalInput").ap()
    b2 = nc.dram_tensor("b2", [Z], F32, kind="ExternalInput").ap()
    pw = nc.dram_tensor("pw", [Z, E], F32, kind="ExternalInput").ap()
    pb = nc.dram_tensor("pb", [E], F32, kind="ExternalInput").ap()
    cb2t = nc.dram_tensor("cb2t", [E, K], F32R, kind="ExternalInput").ap()
    cnorm = nc.dram_tensor("cnorm", [K], F32, kind="ExternalInput").ap()
    cb = nc.dram_tensor("cb", [K, E], F32, kind="ExternalInput").ap()
    d1 = nc.dram_tensor("d1", [E, H], BF16, kind="ExternalInput").ap()
    db1 = nc.dram_tensor("db1", [H], F32, kind="ExternalInput").ap()
    d2 = nc.dram_tensor("d2", [H, X], BF16, kind="ExternalInput").ap()
    db2 = nc.dram_tensor("db2", [X], F32, kind="ExternalInput").ap()

    xrec = nc.dram_tensor("xrec", [BSH, X], F32, kind="ExternalOutput").ap()
    idxo = nc.dram_tensor("idxo", [BSH], U32, kind="ExternalOutput").ap()
    losso = nc.dram_tensor("losso", [P, NBLK * NBT], F32,
                           kind="ExternalOutput").ap()

    ht_dram = nc.dram_tensor("ht_dram", [H, BSH], F32).ap()

    XO, HO, ZO, EO, KCC = X // P, H // P, Z // P, E // P, K // 512

    with TileContext(nc) as tc:
        with (
            tc.tile_pool(name="const", bufs=1) as const_pool,
            tc.tile_pool(name="wstream", bufs=3) as wpool,
            tc.tile_pool(name="astream", bufs=3) as apool,
            tc.tile_pool(name="zbuf", bufs=1) as zpool,
            tc.tile_pool(name="hd", bufs=1) as hdpool,
            tc.tile_pool(name="small", bufs=2) as spool,
            tc.tile_pool(name="cbstream", bufs=3) as cbpool,
            tc.tile_pool(name="tiny", bufs=4) as tpool,
            tc.tile_pool(name="psum", bufs=8, space="PSUM") as pp,
        ):
            ident = const_pool.tile([P, P], F32, tag="ident")
            make_identity(nc, ident)
            b1_sb = const_pool.tile([P, HO], F32, tag="b1")
            nc.sync.dma_start(b1_sb, b1.rearrange("(o p) -> p o", p=P))
            b2_sb = const_pool.tile([P, ZO], F32, tag="b2")
            nc.sync.dma_start(b2_sb, b2.rearrange("(o p) -> p o", p=P))
            pb_sb = const_pool.tile([P, EO], F32, tag="pb")
            nc.sync.dma_start(pb_sb, pb.rearrange("(o p) -> p o", p=P))
            db1_sb = const_pool.tile([P, HO], F32, tag="db1")
            nc.sync.dma_start(db1_sb, db1.rearrange("(o p) -> p o", p=P))
            pw_sb = const_pool.tile([P, ZO, E], F32, tag="pw")
            nc.sync.dma_start(pw_sb, pw.rearrange("(ko p) n -> p ko n", p=P))
            ci512 = const_pool.tile([P, 512], U32, tag="ci512")
            nc.gpsimd.iota(ci512, pattern=[[1, 512]], base=0,
                           channel_multiplier=0)
            c8191 = const_pool.tile([P, 512], U32, tag="c8191")
            nc.vector.memset(c8191, 8191)
            ciob = const_pool.tile([P, 512], U32, tag="ciob")
            nc.vector.tensor_tensor(ciob, c8191, ci512, op=ALU.subtract)

            st = [dict() for _ in range(NBLK)]   # per-block live tiles

            def bslice(b):
                return slice(b * BBLK, (b + 1) * BBLK)

            def enc1_mg(b, mg):
                bsl = bslice(b)
                if True:
                    psums = []
                    for k in range(XO):
                        w1s = wpool.tile([P, 512], F32, tag="w1s")
                        nc.sync.dma_start(
                            w1s, w1[k * P:(k + 1) * P, mg * 512:(mg + 1) * 512])
                        xtc = apool.tile([P, BBLK], F32, tag="xtc")
                        nc.sync.dma_start(xtc, xt[k * P:(k + 1) * P, bsl])
                        if k == 0:
                            psums = [pp.tile([P, BBLK], F32, tag="ps",
                                              name=f"ps_e1_{i}")
                                     for i in range(4)]
                        for m in range(4):
                            nc.tensor.matmul(
                                psums[m], lhsT=w1s[:, m * P:(m + 1) * P],
                                rhs=xtc, start=(k == 0), stop=(k == XO - 1))
                    for m in range(4):
                        hmi = mg * 4 + m
                        hb = apool.tile([P, BBLK], F32, tag="hb")
                        nc.scalar.activation(hb, psums[m], AF.Relu,
                                             bias=b1_sb[:, hmi:hmi + 1])
                        nc.sync.dma_start(
                            ht_dram[hmi * P:(hmi + 1) * P, bsl], hb)

            def enc1(b):
                for mg in range(8):           # groups of 4 H-tiles
                    enc1_mg(b, mg)

            def enc2_pre(b):
                bsl = bslice(b)
                z_t = zpool.tile([P, ZO, BBLK], F32, tag="zt")
                zps = []
                for k in range(HO):
                    w2s = wpool.tile([P, Z], F32, tag="w2s")
                    nc.sync.dma_start(w2s, w2[k * P:(k + 1) * P, :])
                    htc = apool.tile([P, BBLK], F32, tag="htc")
                    nc.sync.dma_start(htc, ht_dram[k * P:(k + 1) * P, bsl])
                    if k == 0:
                        zps = [pp.tile([P, BBLK], F32, tag="ps",
                                       name=f"ps_e2_{i}")
                               for i in range(ZO)]
                    for m in range(ZO):
                        nc.tensor.matmul(zps[m], lhsT=w2s[:, m * P:(m + 1) * P],
                                         rhs=htc, start=(k == 0),
                                         stop=(k == HO - 1))
                for m in range(ZO):
                    nc.scalar.activation(z_t[:, m, :], zps[m], AF.Identity,
                                         bias=b2_sb[:, m:m + 1])

                ze_t = zpool.tile([P, EO, BBLK], F32, tag="zet")
                eps = [pp.tile([P, BBLK], F32, tag="ps", name=f"ps_pre_{i}")
                       for i in range(EO)]
                for k in range(ZO):
                    for m in range(EO):
                        nc.tensor.matmul(
                            eps[m], lhsT=pw_sb[:, k, m * P:(m + 1) * P],
                            rhs=z_t[:, k, :], start=(k == 0),
                            stop=(k == ZO - 1))
                for m in range(EO):
                    nc.scalar.activation(ze_t[:, m, :], eps[m], AF.Identity,
                                         bias=pb_sb[:, m:m + 1])
                st[b]["ze_t"] = ze_t

            def dist_setup(b):
                # coarse f32r scores; pack (score, index) into uint32 so
                # per-chunk MAX8 alone carries the candidates:
                #   pack = trunc((2z.c - |c|^2)*256 + 1024)*8192 + (8191-idx)
                ze_t = st[b]["ze_t"]
                ze_tr = zpool.tile([P, EO, BBLK], F32R, tag="zetr")
                nc.vector.tensor_copy(ze_tr, ze_t)
                # batch-major z_e for the refine, done early to keep the PE
                # ahead of the DVE/gather chain
                ze_bm = [zpool.tile([P, E], F32, tag=f"zebm{bt}",
                                    name=f"zebm{bt}")
                         for bt in range(NBT)]
                for bt in range(NBT):
                    for e in range(EO):
                        tps = pp.tile([P, P], F32, tag="ps")
                        nc.tensor.transpose(
                            tps, ze_t[:, e, bt * P:(bt + 1) * P], ident)
                        nc.vector.tensor_copy(
                            ze_bm[bt][:, e * P:(e + 1) * P], tps)
                st[b]["ze_bm"] = ze_bm
                st[b]["ze_tr"] = ze_tr
                candv = [zpool.tile([P, KCC * 8], U32, tag=f"cv{bt}",
                                    name=f"cv{bt}")
                         for bt in range(NBT)]
                st[b]["candv"] = candv

            def dist_cc(b, cc):
                ze_tr = st[b]["ze_tr"]
                candv = st[b]["candv"]
                if True:
                    ccsl = slice(cc * 512, (cc + 1) * 512)
                    cbc = cbpool.tile([P, EO, 512], F32R, tag="cbc")
                    nc.sync.dma_start(
                        cbc, cb2t.rearrange("(ko p) n -> p ko n", p=P)[:, :, ccsl])
                    cnbc = spool.tile([P, 512], F32, tag="cnbc")
                    nc.sync.dma_start(
                        cnbc,
                        cnorm[ccsl].rearrange("(o n) -> o n", o=1)
                        .broadcast_to([P, 512]))
                    cio = spool.tile([P, 512], U32, tag="cio")
                    nc.vector.tensor_scalar(
                        cio, ciob, 512 * cc, None, op0=ALU.subtract)
                    for bt in range(NBT):
                        dps = pp.tile([P, 512], F32, tag="ps")
                        for e in range(EO):
                            nc.tensor.matmul(
                                dps, lhsT=ze_tr[:, e, bt * P:(bt + 1) * P],
                                rhs=cbc[:, e, :], start=(e == 0),
                                stop=(e == EO - 1))
                        # host pre-scales: psum = 256*(2 z.c), cnbc = 1024-256|c|^2
                        # add + truncate-to-uint in one DVE op
                        su = spool.tile([P, 512], U32, tag="su")
                        nc.vector.tensor_tensor(su, dps, cnbc, op=ALU.add)
                        pk2 = spool.tile([P, 512], U32, tag="pk2")
                        nc.vector.scalar_tensor_tensor(
                            pk2, su, 8192, cio, op0=ALU.mult, op1=ALU.add)
                        nc.vector.max(out=candv[bt][:, cc * 8:(cc + 1) * 8],
                                      in_=pk2)

            def refine(b):
                # exact fp32 re-score of the top-4 candidates per row
                ze_bm = st[b]["ze_bm"]
                candv = st[b]["candv"]
                zq_tb = zpool.tile([P, EO, BBLK], BF16, tag="zqtb")
                idxs, cjs, run_vals, run_idxs = [], [], [], []
                for bt in range(NBT):
                    top8 = tpool.tile([P, 8], U32, tag=f"top8{bt}",
                                      name=f"top8{bt}")
                    nc.vector.max(out=top8, in_=candv[bt])
                    idx4 = tpool.tile([P, 4], U32, tag=f"idx4{bt}",
                                      name=f"idx4{bt}")
                    nc.vector.tensor_scalar(idx4, top8[:, 0:4], 8191, 8191,
                                            op0=ALU.bitwise_and,
                                            op1=ALU.bitwise_xor)
                    idxs.append(idx4)
                for bt in range(NBT):
                    row = []
                    for j in range(4):
                        cj = apool.tile([P, E], F32, tag="zq")
                        nc.gpsimd.indirect_dma_start(
                            out=cj, out_offset=None, in_=cb[:],
                            in_offset=bass.IndirectOffsetOnAxis(
                                ap=idxs[bt][:, j:j + 1], axis=0))
                        row.append(cj)
                    cjs.append(row)
                for bt in range(NBT):
                    run_val = tpool.tile([P, 1], F32, tag=f"rv{bt}",
                                         name=f"rv{bt}")
                    run_idx = tpool.tile([P, 1], U32, tag=f"ri{bt}",
                                         name=f"ri{bt}")
                    nc.vector.memset(run_val, -3.0e38)
                    nc.vector.memset(run_idx, 0)
                    run_vals.append(run_val)
                    run_idxs.append(run_idx)
                    for j in range(4):
                        cj = cjs[bt][j]
                        idx_j = idxs[bt][:, j:j + 1]
                        # exact score = sum(c * (2z - c))
                        t1 = spool.tile([P, E], F32, tag="t1")
                        nc.vector.scalar_tensor_tensor(
                            t1, ze_bm[bt], 2.0, cj, op0=ALU.mult,
                            op1=ALU.subtract)
                        t2 = spool.tile([P, E], F32, tag="t2")
                        nc.vector.tensor_tensor(t2, t1, cj, op=ALU.mult)
                        exj = tpool.tile([P, 1], F32, tag="exj")
                        nc.vector.reduce_sum(exj, t2, axis=mybir.AxisListType.X)
                        m_gt = tpool.tile([P, 1], U32, tag="mgt")
                        nc.vector.tensor_tensor(m_gt, exj, run_val,
                                                op=ALU.is_gt)
                        m_eq = tpool.tile([P, 1], U32, tag="meq")
                        nc.vector.tensor_tensor(m_eq, exj, run_val,
                                                op=ALU.is_equal)
                        m_lt = tpool.tile([P, 1], U32, tag="mlt")
                        nc.vector.tensor_tensor(m_lt, idx_j, run_idx,
                                                op=ALU.is_lt)
                        m_tie = tpool.tile([P, 1], U32, tag="mtie")
                        nc.vector.tensor_tensor(m_tie, m_eq, m_lt,
                                                op=ALU.logical_and)
                        upd = tpool.tile([P, 1], U32, tag="upd")
                        nc.vector.tensor_tensor(upd, m_gt, m_tie,
                                                op=ALU.logical_or)
                        nc.vector.copy_predicated(run_val, upd, exj)
                        nc.vector.copy_predicated(run_idx, upd, idx_j)

                zq_bt = []
                for bt in range(NBT):
                    gbase = b * BBLK + bt * P
                    nc.sync.dma_start(
                        idxo[gbase:gbase + P].rearrange("(n o) -> n o", o=1),
                        run_idxs[bt])
                    zq = apool.tile([P, E], F32, tag="zq")
                    nc.gpsimd.indirect_dma_start(
                        out=zq, out_offset=None, in_=cb[:],
                        in_offset=bass.IndirectOffsetOnAxis(
                            ap=run_idxs[bt][:, 0:1], axis=0))
                    zq_bt.append(zq)
                    # loss partials: sum((z_q - z_e)^2) over this row tile
                    ld = spool.tile([P, E], F32, tag="t1")
                    nc.vector.tensor_sub(ld, zq, ze_bm[bt])
                    lsq = spool.tile([P, E], F32, tag="t2")
                    lacc = tpool.tile([P, 1], F32, tag="lacc")
                    nc.scalar.activation(lsq, ld, AF.Square, accum_out=lacc)
                    nc.sync.dma_start(
                        losso[:, b * NBT + bt:b * NBT + bt + 1], lacc)
                for bt in range(NBT):
                    for e in range(EO):
                        tps = pp.tile([P, P], F32, tag="ps")
                        nc.tensor.transpose(
                            tps, zq_bt[bt][:, e * P:(e + 1) * P], ident)
                        nc.vector.tensor_copy(
                            zq_tb[:, e, bt * P:(bt + 1) * P], tps)
                st[b]["zq_tb"] = zq_tb

            def dec1(b):
                zq_tb = st[b]["zq_tb"]
                hd_t = hdpool.tile([P, HO, BBLK], BF16, tag="hdt")
                for mg in range(8):
                    d1s = wpool.tile([P, EO, 512], BF16, tag="d1s")
                    nc.sync.dma_start(
                        d1s, d1.rearrange("(ko p) n -> p ko n", p=P)
                        [:, :, mg * 512:(mg + 1) * 512])
                    for m in range(4):
                        hmi = mg * 4 + m
                        dps = pp.tile([P, BBLK], F32, tag="ps")
                        for k in range(EO):
                            nc.tensor.matmul(
                                dps, lhsT=d1s[:, k, m * P:(m + 1) * P],
                                rhs=zq_tb[:, k, :], start=(k == 0),
                                stop=(k == EO - 1))
                        nc.scalar.activation(hd_t[:, hmi, :], dps, AF.Relu,
                                             bias=db1_sb[:, hmi:hmi + 1])
                st[b]["hd_t"] = hd_t

            def dec2_xo(b, xo):
                hd_t = st[b]["hd_t"]
                if True:
                    xsl = slice(xo * 512, (xo + 1) * 512)
                    b2bc = spool.tile([P, 512], F32, tag="b2bc")
                    nc.sync.dma_start(
                        b2bc,
                        db2[xsl].rearrange("(o n) -> o n", o=1)
                        .broadcast_to([P, 512]))
                    ops = [pp.tile([P, 512], F32, tag="ps",
                                   name=f"ps_d2_{i}")
                           for i in range(NBT)]
                    for ho in range(HO):
                        w2c = wpool.tile([P, 512], BF16, tag="w2c")
                        nc.sync.dma_start(w2c, d2[ho * P:(ho + 1) * P, xsl])
                        for bt in range(NBT):
                            nc.tensor.matmul(
                                ops[bt], lhsT=hd_t[:, ho, bt * P:(bt + 1) * P],
                                rhs=w2c, start=(ho == 0), stop=(ho == HO - 1))
                    for bt in range(NBT):
                        osb = apool.tile([P, 512], F32, tag="osb")
                        nc.vector.tensor_tensor(osb, ops[bt], b2bc, op=ALU.add)
                        rbase = b * BBLK + bt * P
                        nc.sync.dma_start(xrec[rbase:rbase + P, xsl], osb)

            def dec2(b):
                for xo in range(8):
                    dec2_xo(b, xo)

            # software pipeline over the two blocks: keep the in-order PE fed
            # during the refine's gather/DVE latency
            enc1(0)
            enc2_pre(0)
            dist_setup(0)
            for mg in range(8):
                if mg < 2:
                    for cc in range(8 * mg, 8 * mg + 8):
                        dist_cc(0, cc)
                enc1_mg(1, mg)
            refine(0)
            dec1(0)
            enc2_pre(1)
            dist_setup(1)
            for xo in range(8):
                if xo < 2:
                    for cc in range(8 * xo, 8 * xo + 8):
                        dist_cc(1, cc)
                dec2_xo(0, xo)
            refine(1)
            dec1(1)
            dec2(1)

    nc.compile()
    return nc


def kernel(**inputs):
    inp = {k: np.asarray(v) for k, v in inputs.items()}
    x = inp["x"].astype(np.float32, copy=False)
    codebook = inp["codebook"].astype(np.float32, copy=False)

    if "nc" not in _CACHE:
        _CACHE["nc"] = _build_program()
    nc = _CACHE["nc"]

    cb2t = np.ascontiguousarray((512.0 * codebook).T)
    cnorm = (1024.0 - 256.0 * (codebook.astype(np.float64) ** 2).sum(axis=1)).astype(np.float32)
    d1_bf = inp["dec_w1"].astype(ml_dtypes.bfloat16)
    d2_bf = inp["dec_w2"].astype(ml_dtypes.bfloat16)

    shared = {
        "w1": np.ascontiguousarray(inp["enc_w1"], dtype=np.float32),
        "b1": inp["enc_b1"].astype(np.float32, copy=False),
        "w2": np.ascontiguousarray(inp["enc_w2"], dtype=np.float32),
        "b2": inp["enc_b2"].astype(np.float32, copy=False),
        "pw": np.ascontiguousarray(inp["pre_w"], dtype=np.float32),
        "pb": inp["pre_b"].astype(np.float32, copy=False),
        "cb2t": cb2t,
        "cnorm": cnorm,
        "cb": np.ascontiguousarray(codebook),
        "d1": d1_bf,
        "db1": inp["dec_b1"].astype(np.float32, copy=False),
        "d2": d2_bf,
        "db2": inp["dec_b2"].astype(np.float32, copy=False),
    }
    in_maps = []
    for c in range(NCORES):
        xt_sh = np.ascontiguousarray(x[c * BSH:(c + 1) * BSH].T)
        in_maps.append({**shared, "xt": xt_sh})

    trace = bool(os.environ.get("BASS_TRACE"))
    results = run_bass_kernel_spmd(
        nc, in_maps, list(range(NCORES)), trace=trace,
        tmpdir=os.environ.get("KERNEL_TRACE_DIR") or None)
    _CACHE["last_results"] = results

    x_recon = np.empty((B, X), dtype=np.float32)
    idx_all = np.empty((B,), dtype=np.int64)
    loss_total = 0.0
    for c in range(NCORES):
        r = results.results[c]
        x_recon[c * BSH:(c + 1) * BSH] = r["xrec"]
        idx_all[c * BSH:(c + 1) * BSH] = r["idxo"].astype(np.int64)
        loss_total += r["losso"].astype(np.float64).sum()

    mse = loss_total / (B * E)
    vq_loss = np.float32((1.0 + COMMIT) * mse)

    counts = np.bincount(idx_all, minlength=K).astype(np.float32)
    avg = counts / np.float32(B)
    perplexity = np.float32(
        np.exp(-np.sum(avg * np.log(avg + 1e-10), dtype=np.float64)))

    return (x_recon, vq_loss, perplexity)
